# revision 16
# baseline (speedup 1.0000x reference)
"""Multi-head attention (B=4, T=2048, C=1024, H=16, D=64) on 8 TRN2 NeuronCores.

Sharding: data-parallel over the 4 batches x tensor-parallel over 2 head
groups (8 heads each).  Core c handles batch (c % 4), head group (c // 4).

Per-core kernel (all matmuls in bf16, fp32 accumulation):
  qT = (Wq_g x_b^T + bq_g)        [512, 2048]  (c_out on partitions)
  kT = (Wk_g x_b^T + bk_g)        [512, 2048]
  v  = (x_b Wv_g^T)               [2048, 512]  (t on partitions; bv folded on host)
  per head h:  S^T = kT_h^T-contraction: scoresT[tk, tq] (K=64 matmuls)
               P = exp(S^T / 8)   (scalar engine, PSUM -> SBUF bf16)
               A^T[d, tq] = sum_tk [V_h | 1] P  (M=65: row 64 = softmax sums)
               A_h = A^T[0:64] * (1/sums)  (PE broadcast + DVE mul)
  oT_partial = Wo_g^T-contraction over the 8 heads  [1024, 2048] fp32 -> HBM

Host: out[b] = (oT(b, g0) + oT(b, g1)).T + bo + Wo @ bv
(the V-bias contributes exactly Wo @ bv per row because softmax rows sum to 1).
"""

import sys

if "/opt/trn_rl_repo" not in sys.path:
    sys.path.insert(0, "/opt/trn_rl_repo")

import numpy as np
import ml_dtypes

from concourse.bacc import Bacc
import concourse.mybir as mybir
import concourse.tile as tile
from concourse.bass_utils import run_bass_kernel_spmd

F32 = mybir.dt.float32
F32R = mybir.dt.float32r
BF16 = mybir.dt.float16  # compute dtype (fp16: same PE speed as bf16, more mantissa)
EXPF = mybir.ActivationFunctionType.Exp

B, T, C = 4, 2048, 1024
H, D = 16, 64
HPC = 8          # heads per core
CS = HPC * D     # c_out slice per core = 512
NKT = T // 128   # 16 k-tiles over t_k
NQC = T // 512   # 4 q-chunks of 512
P_BUFS = 24


def build_nc():
    nc = Bacc(trn_type="TRN2")
    xT_d = nc.dram_tensor("xT", [C, T], BF16, kind="ExternalInput")
    wq_d = nc.dram_tensor("wqT", [C, CS], BF16, kind="ExternalInput")
    wk_d = nc.dram_tensor("wkT", [C, CS], BF16, kind="ExternalInput")
    wv_d = nc.dram_tensor("wvT", [C, CS], BF16, kind="ExternalInput")
    wo_d = nc.dram_tensor("woT", [CS, C], BF16, kind="ExternalInput")
    bq_d = nc.dram_tensor("bq", [CS, 1], F32, kind="ExternalInput")
    bk_d = nc.dram_tensor("bk", [CS, 1], F32, kind="ExternalInput")
    oT_d = nc.dram_tensor("oT", [C, T], F32, kind="ExternalOutput")

    with tile.TileContext(nc) as tc:
        with (
            tc.tile_pool(name="consts", bufs=1) as consts,
            tc.tile_pool(name="qkv", bufs=1) as qkv,
            tc.tile_pool(name="ps", bufs=2, space="PSUM") as ps,
        ):
            # ---- persistent tiles ----
            wo_sb = consts.tile([128, 4, C], BF16)
            nc.sync.dma_start(out=wo_sb, in_=wo_d[:, :].rearrange("(c p) n -> p c n", p=128))
            bq_sb = consts.tile([128, 4], F32)
            nc.sync.dma_start(out=bq_sb, in_=bq_d[:, :].rearrange("(c p) n -> p (c n)", p=128))
            bk_sb = consts.tile([128, 4], F32)
            nc.sync.dma_start(out=bk_sb, in_=bk_d[:, :].rearrange("(c p) n -> p (c n)", p=128))

            qT_sb = qkv.tile([128, 4, T], BF16)
            kT_sb = qkv.tile([128, 4, T], BF16)
            # V per head with a ones column appended: [tk partition, ktile, head, 64+1]
            vh_sb = qkv.tile([128, NKT, HPC, D + 1], BF16)
            nc.vector.memset(vh_sb[:, :, :, D:D + 1], 1.0)
            a_sb = qkv.tile([128, 4, T], BF16)

            # ---- QKV-phase tiles (pool closed after QKV so P tiles reuse it) --
            xw = tc.tile_pool(name="xw", bufs=1)
            xwp = xw.__enter__()
            xT_sb = xwp.tile([128, 8, T], BF16)
            nc.sync.dma_start(out=xT_sb, in_=xT_d[:, :].rearrange("(c p) n -> p c n", p=128))
            wq_sb = xwp.tile([128, 8, CS], BF16)
            nc.sync.dma_start(out=wq_sb, in_=wq_d[:, :].rearrange("(c p) n -> p c n", p=128))
            wk_sb = xwp.tile([128, 8, CS], BF16)
            nc.sync.dma_start(out=wk_sb, in_=wk_d[:, :].rearrange("(c p) n -> p c n", p=128))
            wv_sb = xwp.tile([128, 8, CS], BF16)
            nc.sync.dma_start(out=wv_sb, in_=wv_d[:, :].rearrange("(c p) n -> p c n", p=128))

            # ---- QKV projections ----
            # q/k: out[c_out, t]; lhsT = W^T tile [c_in 128, c_out 128], rhs = xT.
            # ci-middle loop shares each LDWEIGHTS across the 4 t-chunks
            # (4 accumulation groups live in the 4 "acc" PSUM slots).
            for (w_sb, b_sb, dst) in ((wq_sb, bq_sb, qT_sb), (wk_sb, bk_sb, kT_sb)):
                for mt in range(4):
                    pmms = [ps.tile([128, 512], F32, tag="acc", bufs=4, name="pmm")
                            for _ in range(4)]
                    for ci in range(8):
                        for t in range(4):
                            nc.tensor.matmul(
                                pmms[t],
                                w_sb[:, ci, mt * 128:(mt + 1) * 128],
                                xT_sb[:, ci, t * 512:(t + 1) * 512],
                                start=(ci == 0), stop=(ci == 7),
                            )
                    for t in range(4):
                        nc.vector.tensor_scalar_add(
                            dst[:, mt, t * 512:(t + 1) * 512], pmms[t], b_sb[:, mt:mt + 1]
                        )
            # v: out[t, c_out]; lhsT = xT tile [c_in 128, t 128], rhs = wv tile
            for tt in range(NKT):
                pmm = ps.tile([128, 512], F32, tag="acc", bufs=4, name="pmm")
                for ci in range(8):
                    nc.tensor.matmul(
                        pmm,
                        xT_sb[:, ci, tt * 128:(tt + 1) * 128],
                        wv_sb[:, ci, :],
                        start=(ci == 0), stop=(ci == 7),
                    )
                for h in range(HPC):
                    nc.vector.tensor_copy(
                        vh_sb[:, tt, h, 0:D], pmm[:, h * D:(h + 1) * D]
                    )

            xw.__exit__(None, None, None)
            ppool_cm = tc.tile_pool(name="ptiles", bufs=P_BUFS)
            ppool = ppool_cm.__enter__()
            small_cm = tc.tile_pool(name="small", bufs=2)
            small = small_cm.__enter__()
            ostage_cm = tc.tile_pool(name="ostage", bufs=4)
            ostage = ostage_cm.__enter__()

            # ---- attention: head pairs x query halves ----
            # Pass (hp, qh): heads A=2hp (partitions 0:64) and B=2hp+1 (64:128),
            # query range qh*1024..+1024.  Scores for A and B go to one
            # [128, 2048] PSUM tile (A|B) via row-group-paired K=64 matmuls that
            # run concurrently on the PE; one exp covers both heads.
            def attn_pass(hp, qh):
                ptiles = []
                for kt in range(NKT):
                    pt = ppool.tile([128, 2048], BF16, tag="P", bufs=P_BUFS, name="pt")
                    ptiles.append(pt)
                    sc = ps.tile([128, 2048], F32, tag="sc", bufs=1, name="sc")
                    for j in range(2):
                        q0 = qh * 1024 + j * 512
                        for hb in range(2):  # A then B, adjacent for row-group pairing
                            nc.tensor.matmul(
                                sc[:, hb * 1024 + j * 512:hb * 1024 + (j + 1) * 512],
                                kT_sb[hb * 64:hb * 64 + 64, hp, kt * 128:(kt + 1) * 128],
                                qT_sb[hb * 64:hb * 64 + 64, hp, q0:q0 + 512],
                                start=True, stop=True,
                            )
                    nc.scalar.activation(pt, sc, EXPF, scale=0.125)
                avs = [ps.tile([65, 512], F32, tag="acc", bufs=4, name="av")
                       for _ in range(4)]  # [hb*2 + qc2]
                for kt in range(NKT):
                    for hb in range(2):
                        for qc2 in range(2):
                            nc.tensor.matmul(
                                avs[hb * 2 + qc2],
                                vh_sb[:, kt, 2 * hp + hb, :],
                                ptiles[kt][:, hb * 1024 + qc2 * 512:hb * 1024 + (qc2 + 1) * 512],
                                start=(kt == 0), stop=(kt == NKT - 1),
                            )
                for hb in range(2):
                    for qc2 in range(2):
                        av = avs[hb * 2 + qc2]
                        qc = qh * 2 + qc2
                        ssum = small.tile([1, 512], F32, tag="ssum", bufs=1, name="ssum")
                        nc.vector.tensor_copy(ssum, av[64:65, :])
                        rec = small.tile([1, 512], F32, tag="rec", bufs=1, name="rec")
                        nc.vector.reciprocal_approx_fast(out=rec, in_=ssum)
                        rbs = small.tile([64, 512], F32, tag="rbs", bufs=1, name="rbs")
                        nc.gpsimd.partition_broadcast(rbs, rec)
                        nc.vector.tensor_mul(
                            a_sb[hb * 64:hb * 64 + 64, hp, qc * 512:(qc + 1) * 512],
                            av[0:64, :], rbs,
                        )

            def outproj(qh):
                # oT[c_out, t] for the two q-chunks of this half
                for mt in range(8):
                    pos = [ps.tile([128, 512], F32, tag="acc", bufs=4, name="po")
                           for _ in range(2)]
                    for ci in range(4):
                        for qc2 in range(2):
                            qc = qh * 2 + qc2
                            nc.tensor.matmul(
                                pos[qc2],
                                wo_sb[:, ci, mt * 128:(mt + 1) * 128],
                                a_sb[:, ci, qc * 512:(qc + 1) * 512],
                                start=(ci == 0), stop=(ci == 3),
                            )
                    for qc2 in range(2):
                        qc = qh * 2 + qc2
                        ot = ostage.tile([128, 512], F32, tag="ot", bufs=3, name="ot")
                        nc.vector.tensor_copy(ot, pos[qc2])
                        nc.sync.dma_start(
                            out=oT_d[mt * 128:(mt + 1) * 128, qc * 512:(qc + 1) * 512],
                            in_=ot,
                        )

            for qh in range(2):
                for hp in range(4):
                    attn_pass(hp, qh)
                outproj(qh)
            ostage_cm.__exit__(None, None, None)
            small_cm.__exit__(None, None, None)
            ppool_cm.__exit__(None, None, None)
    nc.finalize()
    return nc


_NC = None


def _get_nc():
    global _NC
    if _NC is None:
        _NC = build_nc()
    return _NC


def _shard_inputs(x, Wq, bq, Wk, bk, Wv, bv, Wo, bo):
    bf = np.float16
    x = np.asarray(x, np.float32)
    in_maps = []
    wqT = np.ascontiguousarray(np.asarray(Wq, np.float32).T).astype(bf)  # [C, C] = [c_in, c_out]
    wkT = np.ascontiguousarray(np.asarray(Wk, np.float32).T).astype(bf)
    wvT = np.ascontiguousarray(np.asarray(Wv, np.float32).T).astype(bf)
    woT = np.ascontiguousarray(np.asarray(Wo, np.float32).T).astype(bf)  # [c_in, c_out]
    xT = [np.ascontiguousarray(x[b].T).astype(bf) for b in range(B)]
    for c in range(8):
        b, g = c % B, c // B
        sl = slice(g * CS, (g + 1) * CS)
        in_maps.append({
            "xT": xT[b],
            "wqT": np.ascontiguousarray(wqT[:, sl]),
            "wkT": np.ascontiguousarray(wkT[:, sl]),
            "wvT": np.ascontiguousarray(wvT[:, sl]),
            "woT": np.ascontiguousarray(woT[sl, :]),
            "bq": np.ascontiguousarray(np.asarray(bq, np.float32)[sl]).reshape(CS, 1),
            "bk": np.ascontiguousarray(np.asarray(bk, np.float32)[sl]).reshape(CS, 1),
        })
    return in_maps


def run_sharded(inputs, **kwargs):
    """Run the SPMD kernel; returns the BassKernelResults."""
    nc = _get_nc()
    in_maps = _shard_inputs(**inputs)
    return run_bass_kernel_spmd(nc, in_maps, core_ids=list(range(8)), **kwargs)


def assemble(results, Wv_bias, Wo, bo):
    bo_eff = (np.asarray(bo, np.float32)
              + np.asarray(Wo, np.float32) @ np.asarray(Wv_bias, np.float32))
    out = np.empty((B, T, C), np.float32)
    for b in range(B):
        acc = results[b]["oT"].astype(np.float32) + results[b + B]["oT"].astype(np.float32)
        out[b] = acc.T + bo_eff[None, :]
    return out


def kernel(**inputs):
    res = run_sharded(inputs)
    return assemble(res.results, inputs["bv"], inputs["Wo"], inputs["bo"])


# revision 17
# speedup vs baseline: 1.2959x; 1.2959x over previous
"""Multi-head attention (B=4, T=2048, C=1024, H=16, D=64) on 8 TRN2 NeuronCores.

Sharding: data-parallel over the 4 batches x tensor-parallel over 2 head
groups (8 heads each).  Core c handles batch (c % 4), head group (c // 4).

Per-core kernel (all matmuls in bf16, fp32 accumulation):
  qT = (Wq_g x_b^T + bq_g)        [512, 2048]  (c_out on partitions)
  kT = (Wk_g x_b^T + bk_g)        [512, 2048]
  v  = (x_b Wv_g^T)               [2048, 512]  (t on partitions; bv folded on host)
  per head h:  S^T = kT_h^T-contraction: scoresT[tk, tq] (K=64 matmuls)
               P = exp(S^T / 8)   (scalar engine, PSUM -> SBUF bf16)
               A^T[d, tq] = sum_tk [V_h | 1] P  (M=65: row 64 = softmax sums)
               A_h = A^T[0:64] * (1/sums)  (PE broadcast + DVE mul)
  oT_partial = Wo_g^T-contraction over the 8 heads  [1024, 2048] fp32 -> HBM

Host: out[b] = (oT(b, g0) + oT(b, g1)).T + bo + Wo @ bv
(the V-bias contributes exactly Wo @ bv per row because softmax rows sum to 1).
"""

import sys

if "/opt/trn_rl_repo" not in sys.path:
    sys.path.insert(0, "/opt/trn_rl_repo")

import numpy as np
import ml_dtypes

from concourse.bacc import Bacc
import concourse.mybir as mybir
import concourse.tile as tile
from concourse.bass_utils import run_bass_kernel_spmd

F32 = mybir.dt.float32
F32R = mybir.dt.float32r
BF16 = mybir.dt.float16  # compute dtype (fp16: same PE speed as bf16, more mantissa)
EXPF = mybir.ActivationFunctionType.Exp

B, T, C = 4, 2048, 1024
H, D = 16, 64
HPC = 8          # heads per core
CS = HPC * D     # c_out slice per core = 512
NKT = T // 128   # 16 k-tiles over t_k
NQC = T // 512   # 4 q-chunks of 512
P_BUFS = 24


def build_nc():
    nc = Bacc(trn_type="TRN2")
    xT_d = nc.dram_tensor("xT", [C, T], BF16, kind="ExternalInput")
    wq_d = nc.dram_tensor("wqT", [C, CS], BF16, kind="ExternalInput")
    wk_d = nc.dram_tensor("wkT", [C, CS], BF16, kind="ExternalInput")
    wv_d = nc.dram_tensor("wvT", [C, CS], BF16, kind="ExternalInput")
    wo_d = nc.dram_tensor("woT", [CS, C], BF16, kind="ExternalInput")
    bq_d = nc.dram_tensor("bq", [CS, 1], F32, kind="ExternalInput")
    bk_d = nc.dram_tensor("bk", [CS, 1], F32, kind="ExternalInput")
    oT_d = nc.dram_tensor("oT", [C, T], F32, kind="ExternalOutput")

    with tile.TileContext(nc) as tc:
        with (
            tc.tile_pool(name="consts", bufs=1) as consts,
            tc.tile_pool(name="qkv", bufs=1) as qkv,
            tc.tile_pool(name="ps", bufs=2, space="PSUM") as ps,
        ):
            # ---- persistent tiles ----
            wo_sb = consts.tile([128, 4, C], BF16)
            nc.sync.dma_start(out=wo_sb, in_=wo_d[:, :].rearrange("(c p) n -> p c n", p=128))
            bq_sb = consts.tile([128, 4], F32)
            nc.sync.dma_start(out=bq_sb, in_=bq_d[:, :].rearrange("(c p) n -> p (c n)", p=128))
            bk_sb = consts.tile([128, 4], F32)
            nc.sync.dma_start(out=bk_sb, in_=bk_d[:, :].rearrange("(c p) n -> p (c n)", p=128))

            qT_sb = qkv.tile([128, 4, T], BF16)
            kT_sb = qkv.tile([128, 4, T], BF16)
            # V per head with a ones column appended: [tk partition, ktile, head, 64+1]
            vh_sb = qkv.tile([128, NKT, HPC, D + 1], BF16)
            nc.vector.memset(vh_sb[:, :, :, D:D + 1], 1.0)
            a_sb = qkv.tile([128, 4, T], BF16)

            # ---- QKV-phase tiles (pool closed after QKV so P tiles reuse it) --
            xw = tc.tile_pool(name="xw", bufs=1)
            xwp = xw.__enter__()
            xT_sb = xwp.tile([128, 8, T], BF16)
            nc.sync.dma_start(out=xT_sb, in_=xT_d[:, :].rearrange("(c p) n -> p c n", p=128))
            wq_sb = xwp.tile([128, 8, CS], BF16)
            nc.sync.dma_start(out=wq_sb, in_=wq_d[:, :].rearrange("(c p) n -> p c n", p=128))
            wk_sb = xwp.tile([128, 8, CS], BF16)
            nc.sync.dma_start(out=wk_sb, in_=wk_d[:, :].rearrange("(c p) n -> p c n", p=128))
            wv_sb = xwp.tile([128, 8, CS], BF16)
            nc.sync.dma_start(out=wv_sb, in_=wv_d[:, :].rearrange("(c p) n -> p c n", p=128))

            # ---- QKV projections ----
            # q/k: out[c_out, t]; lhsT = W^T tile [c_in 128, c_out 128], rhs = xT.
            # ci-middle loop shares each LDWEIGHTS across the 4 t-chunks
            # (4 accumulation groups live in the 4 "acc" PSUM slots).
            for (w_sb, b_sb, dst) in ((wq_sb, bq_sb, qT_sb), (wk_sb, bk_sb, kT_sb)):
                for mt in range(4):
                    pmms = [ps.tile([128, 512], F32, tag="acc", bufs=4, name="pmm")
                            for _ in range(4)]
                    for ci in range(8):
                        for t in range(4):
                            nc.tensor.matmul(
                                pmms[t],
                                w_sb[:, ci, mt * 128:(mt + 1) * 128],
                                xT_sb[:, ci, t * 512:(t + 1) * 512],
                                start=(ci == 0), stop=(ci == 7),
                            )
                    for t in range(4):
                        nc.vector.tensor_scalar_add(
                            dst[:, mt, t * 512:(t + 1) * 512], pmms[t], b_sb[:, mt:mt + 1]
                        )
            # v: out[t, c_out]; lhsT = xT tile [c_in 128, t 128], rhs = wv tile
            for tt in range(NKT):
                pmm = ps.tile([128, 512], F32, tag="acc", bufs=4, name="pmm")
                for ci in range(8):
                    nc.tensor.matmul(
                        pmm,
                        xT_sb[:, ci, tt * 128:(tt + 1) * 128],
                        wv_sb[:, ci, :],
                        start=(ci == 0), stop=(ci == 7),
                    )
                for h in range(HPC):
                    nc.vector.tensor_copy(
                        vh_sb[:, tt, h, 0:D], pmm[:, h * D:(h + 1) * D]
                    )

            xw.__exit__(None, None, None)
            ppool_cm = tc.tile_pool(name="ptiles", bufs=P_BUFS)
            ppool = ppool_cm.__enter__()
            small_cm = tc.tile_pool(name="small", bufs=2)
            small = small_cm.__enter__()
            ostage_cm = tc.tile_pool(name="ostage", bufs=4)
            ostage = ostage_cm.__enter__()

            # ---- attention: head pairs x query halves ----
            # Pass (hp, qh): heads A=2hp (partitions 0:64) and B=2hp+1 (64:128),
            # query range qh*1024..+1024.  Scores for A and B go to one
            # [128, 2048] PSUM tile (A|B) via row-group-paired K=64 matmuls that
            # run concurrently on the PE; one exp covers both heads.
            def attn_pass(hp, qh):
                # P tile layout per kt: [A(j0) | B(j0) | A(j1) | B(j1)], 512 each
                ptiles = []
                for kt in range(NKT):
                    pt = ppool.tile([128, 2048], BF16, tag="P", bufs=P_BUFS, name="pt")
                    ptiles.append(pt)
                    for j in range(2):
                        sc = ps.tile([128, 1024], F32, tag="sc", bufs=2, name="sc")
                        q0 = qh * 1024 + j * 512
                        for hb in range(2):  # A then B, adjacent for row-group pairing
                            nc.tensor.matmul(
                                sc[:, hb * 512:(hb + 1) * 512],
                                kT_sb[hb * 64:hb * 64 + 64, hp, kt * 128:(kt + 1) * 128],
                                qT_sb[hb * 64:hb * 64 + 64, hp, q0:q0 + 512],
                                start=True, stop=True,
                            )
                        nc.scalar.activation(
                            pt[:, j * 1024:(j + 1) * 1024], sc, EXPF, scale=0.125
                        )
                avs = [ps.tile([65, 512], F32, tag="acc", bufs=4, name="av")
                       for _ in range(4)]  # [hb*2 + qc2]
                for kt in range(NKT):
                    for hb in range(2):
                        for qc2 in range(2):
                            nc.tensor.matmul(
                                avs[hb * 2 + qc2],
                                vh_sb[:, kt, 2 * hp + hb, :],
                                ptiles[kt][:, qc2 * 1024 + hb * 512:qc2 * 1024 + (hb + 1) * 512],
                                start=(kt == 0), stop=(kt == NKT - 1),
                            )
                for hb in range(2):
                    for qc2 in range(2):
                        av = avs[hb * 2 + qc2]
                        qc = qh * 2 + qc2
                        ssum = small.tile([1, 512], F32, tag="ssum", bufs=1, name="ssum")
                        nc.vector.tensor_copy(ssum, av[64:65, :])
                        rec = small.tile([1, 512], F32, tag="rec", bufs=1, name="rec")
                        nc.vector.reciprocal_approx_fast(out=rec, in_=ssum)
                        rbs = small.tile([64, 512], F32, tag="rbs", bufs=1, name="rbs")
                        nc.gpsimd.partition_broadcast(rbs, rec)
                        nc.vector.tensor_mul(
                            a_sb[hb * 64:hb * 64 + 64, hp, qc * 512:(qc + 1) * 512],
                            av[0:64, :], rbs,
                        )

            def outproj(qh):
                # oT[c_out, t] for the two q-chunks of this half
                for mt in range(8):
                    pos = [ps.tile([128, 512], F32, tag="acc", bufs=4, name="po")
                           for _ in range(2)]
                    for ci in range(4):
                        for qc2 in range(2):
                            qc = qh * 2 + qc2
                            nc.tensor.matmul(
                                pos[qc2],
                                wo_sb[:, ci, mt * 128:(mt + 1) * 128],
                                a_sb[:, ci, qc * 512:(qc + 1) * 512],
                                start=(ci == 0), stop=(ci == 3),
                            )
                    for qc2 in range(2):
                        qc = qh * 2 + qc2
                        ot = ostage.tile([128, 512], F32, tag="ot", bufs=3, name="ot")
                        nc.vector.tensor_copy(ot, pos[qc2])
                        nc.sync.dma_start(
                            out=oT_d[mt * 128:(mt + 1) * 128, qc * 512:(qc + 1) * 512],
                            in_=ot,
                        )

            for qh in range(2):
                for hp in range(4):
                    attn_pass(hp, qh)
                outproj(qh)
            ostage_cm.__exit__(None, None, None)
            small_cm.__exit__(None, None, None)
            ppool_cm.__exit__(None, None, None)
    nc.finalize()
    return nc


_NC = None


def _get_nc():
    global _NC
    if _NC is None:
        _NC = build_nc()
    return _NC


def _shard_inputs(x, Wq, bq, Wk, bk, Wv, bv, Wo, bo):
    bf = np.float16
    x = np.asarray(x, np.float32)
    in_maps = []
    wqT = np.ascontiguousarray(np.asarray(Wq, np.float32).T).astype(bf)  # [C, C] = [c_in, c_out]
    wkT = np.ascontiguousarray(np.asarray(Wk, np.float32).T).astype(bf)
    wvT = np.ascontiguousarray(np.asarray(Wv, np.float32).T).astype(bf)
    woT = np.ascontiguousarray(np.asarray(Wo, np.float32).T).astype(bf)  # [c_in, c_out]
    xT = [np.ascontiguousarray(x[b].T).astype(bf) for b in range(B)]
    for c in range(8):
        b, g = c % B, c // B
        sl = slice(g * CS, (g + 1) * CS)
        in_maps.append({
            "xT": xT[b],
            "wqT": np.ascontiguousarray(wqT[:, sl]),
            "wkT": np.ascontiguousarray(wkT[:, sl]),
            "wvT": np.ascontiguousarray(wvT[:, sl]),
            "woT": np.ascontiguousarray(woT[sl, :]),
            "bq": np.ascontiguousarray(np.asarray(bq, np.float32)[sl]).reshape(CS, 1),
            "bk": np.ascontiguousarray(np.asarray(bk, np.float32)[sl]).reshape(CS, 1),
        })
    return in_maps


def run_sharded(inputs, **kwargs):
    """Run the SPMD kernel; returns the BassKernelResults."""
    nc = _get_nc()
    in_maps = _shard_inputs(**inputs)
    return run_bass_kernel_spmd(nc, in_maps, core_ids=list(range(8)), **kwargs)


def assemble(results, Wv_bias, Wo, bo):
    bo_eff = (np.asarray(bo, np.float32)
              + np.asarray(Wo, np.float32) @ np.asarray(Wv_bias, np.float32))
    out = np.empty((B, T, C), np.float32)
    for b in range(B):
        acc = results[b]["oT"].astype(np.float32) + results[b + B]["oT"].astype(np.float32)
        out[b] = acc.T + bo_eff[None, :]
    return out


def kernel(**inputs):
    res = run_sharded(inputs)
    return assemble(res.results, inputs["bv"], inputs["Wo"], inputs["bo"])
